# revision 1
# baseline (speedup 1.0000x reference)
"""CosformerAttention (causal linear attention) Trainium2 Bass kernel.

Full inputs in, full output out. Shards batch*heads over 8 NeuronCores:
device d handles sample n = d//4 and heads hA = 2*(d%4), hB = hA+1.
Per device: q/k/v projections for its 2 heads (bf16 matmuls), chunked
causal linear attention with prefix-summed inter-chunk states, and a
partial output projection over its 128 local features; the host sums
the 4 per-sample partials.

Self-contained: hardcodes L=1024, N=2, E=512, H=8 from the problem spec.
"""

import sys

if "/opt/trn_rl_repo" not in sys.path:
    sys.path.insert(0, "/opt/trn_rl_repo")

import numpy as np
import ml_dtypes

BF16NP = ml_dtypes.bfloat16

import concourse.bass as bass
import concourse.tile as tile
from concourse import mybir
import concourse.bass_utils as bass_utils
from concourse.vector_clock import ScopedClock

F32 = mybir.dt.float32
BF16 = mybir.dt.bfloat16
ALU = mybir.AluOpType
ACTF = mybir.ActivationFunctionType

L, N, E, H = 1024, 2, 512, 8
D = E // H          # 64 head dim
DD = 2 * D          # 128 cos/sin-doubled head dim
P = 128             # partitions / chunk size
NCHUNK = L // P     # 8
NCORES = 8
EPS = 1e-6


# ---------------------------------------------------------------------------
# This walrus build allows at most ONE semaphore wait per instruction.
# (a) Tile's tail drain carries the whole global clock: split it across
#     preceding SP nops.  (b) Skip the tail barriers + semaphore clearing --
#     the Bass preamble already dma_resets + sem_clears the entire kernel
#     semaphore range at program start, so end-of-kernel cleanup is
#     redundant and costs ~10us of EVSEM butterfly.
# ---------------------------------------------------------------------------
def _patched_drain_and_barrier(self, tick_clock, wait_clock):
    nc = self.nc
    nops = [nc.sync.nop() for _ in range(48)]
    drain_inst = nc.sync.drain()
    wait_clock.add_sem_waits(
        drain_inst.ins, ScopedClock({None: tick_clock.global_clock})
    )
    waits = list(drain_inst.ins.sync_info.on_wait or [])
    if len(waits) > 1:
        drain_inst.ins.sync_info.on_wait = [waits[-1]]
        SI = type(drain_inst.ins.sync_info)
        for nop, w in zip(nops, waits[:-1]):
            si = nop.ins.sync_info
            if si is None:
                nop.ins.sync_info = SI(on_wait=[w], on_update=[])
            else:
                si.on_wait = [w]
    nc.all_engine_barrier()
    popped = nc._tile_sem_poison_stack.pop()
    assert popped is self._sem_poison


tile.TileContext._drain_and_barrier = _patched_drain_and_barrier


def _split_multi_waits(nc):
    """Move excess sem waits onto preceding same-engine NoOps (engines
    execute strictly in order, so this is equivalent)."""
    k = 0
    for f in nc.m.functions:
        for bb in f.blocks:
            insts = list(bb.instructions)
            out, changed = [], False
            for inst in insts:
                si = inst.sync_info
                waits = list(si.on_wait) if (si is not None and si.on_wait) else []
                if len(waits) > 1 and "Unassigned" not in str(inst.engine):
                    for w in waits[:-1]:
                        nop = mybir.InstNoOp(name=f"wsplit-{k}", ins=[], outs=[])
                        k += 1
                        nop.engine = inst.engine
                        nop.sync_info = type(si)(on_wait=[w], on_update=[])
                        out.append(nop)
                    si.on_wait = [waits[-1]]
                    changed = True
                out.append(inst)
            if changed:
                bb.instructions = out


def bcast(ap, dims):
    """Append broadcast (step 0) free dims to an AP."""
    return bass.AP(tensor=ap.tensor, offset=ap.offset,
                   ap=list(ap.ap) + [[0, d] for d in dims])


def build_program():
    nc = bass.Bass("TRN2", target_bir_lowering=False)

    # ---- DRAM I/O (packed to minimize DMA trigger count) -------------------
    # xT: (4*128, L) bf16 -- x transposed, e-major
    xT_d = nc.dram_tensor("xT", [E, L], BF16, kind="ExternalInput").ap()
    # w_all: (512, 768) bf16 = [wq_dup (256) | wk_dup (256) | w_vk (256)]
    w_d = nc.dram_tensor("w_all", [E, 768], BF16, kind="ExternalInput").ap()
    # wb16: (128, 640) bf16 = [outwT (512) | ident (128)]
    wb_d = nc.dram_tensor("wb16", [P, 640], BF16, kind="ExternalInput").ap()
    # cf32: (128, 1172) f32 =
    #   [sc_full 0:1024 | mask 1024:1152 | s_col 1152:1160 | c_col 1160:1168 |
    #    qb 1168:1170 | kb 1170:1172]
    cf_d = nc.dram_tensor("cf32", [P, 1172], F32, kind="ExternalInput").ap()
    # row1: (1, 384) bf16 = [vkb (256) | ones (128)]
    row1_d = nc.dram_tensor("row1", [1, 384], BF16, kind="ExternalInput").ap()
    out_d = nc.dram_tensor("out", [L, E], F32, kind="ExternalOutput").ap()

    with tile.TileContext(nc) as tc:
        persist = tc.alloc_tile_pool(name="persist", bufs=1)
        work = tc.alloc_tile_pool(name="work", bufs=3)
        small = tc.alloc_tile_pool(name="small", bufs=4)
        ps_big = tc.alloc_tile_pool(name="ps_big", bufs=2, space="PSUM")
        ps_misc = tc.alloc_tile_pool(name="ps_misc", bufs=1, space="PSUM")
        ps_po = tc.alloc_tile_pool(name="ps_po", bufs=3, space="PSUM")

        # ---- batched input loads ------------------------------------------
        xT = persist.tile([P, 4, L], BF16, tag="xT", name="xT")
        nc.sync.dma_start(out=xT[:], in_=xT_d.rearrange("(e p) l -> p e l", p=P))
        w_all = persist.tile([P, 4, 768], BF16, tag="w_all", name="w_all")
        nc.sync.dma_start(out=w_all[:], in_=w_d.rearrange("(e p) f -> p e f", p=P))
        wb16 = persist.tile([P, 640], BF16, tag="wb16", name="wb16")
        nc.sync.dma_start(out=wb16[:], in_=wb_d)
        cf32 = persist.tile([P, 1172], F32, tag="cf32", name="cf32")
        nc.sync.dma_start(out=cf32[:], in_=cf_d)
        row1 = persist.tile([1, 384], BF16, tag="row1", name="row1")
        nc.sync.dma_start(out=row1[:], in_=row1_d)

        def wq(e):
            return w_all[:, e, 0:256]

        def wk(e):
            return w_all[:, e, 256:512]

        def wvk(e):
            return w_all[:, e, 512:768]

        outw = wb16[:, 0:512]
        ident = wb16[:, 512:640]
        sc = cf32[:, 0:1024]
        mask = cf32[:, 1024:1152]
        scol = cf32[:, 1152:1160]
        ccol = cf32[:, 1160:1168]
        vkb = row1[:, 0:256]
        ones_row = row1[:, 256:384]

        # persistent activations
        q_f = [persist.tile([P, L], BF16, tag=f"qf{h}", name=f"qf{h}") for h in range(2)]
        k_f = [persist.tile([P, L], BF16, tag=f"kf{h}", name=f"kf{h}") for h in range(2)]
        # k_t: [ch, head, sc, d] sequence-layout scaled k
        k_t = persist.tile([P, NCHUNK, 2, 2, D], BF16, tag="kt", name="kt")
        # v_t: [ch, head, d+1] with ones column
        v_t = persist.tile([P, NCHUNK, 2, D + 1], BF16, tag="vt", name="vt")
        attn = persist.tile([P, NCHUNK, P], BF16, tag="attn", name="attn")
        Sc_sb = persist.tile([P, NCHUNK, 2, D + 1], BF16, tag="scsb", name="scsb")
        Spfx = persist.tile([P, NCHUNK, 2, D + 1], BF16, tag="spfx", name="spfx")
        aT = persist.tile([P, NCHUNK, P], BF16, tag="aT", name="aT")

        # ---- stage B: feature-layout q_/k_ ((2d, L), scaled by sin/cos) ----
        for si in range(4):
            wsel = wq if si < 2 else wk
            bcol = 1168 + si  # qbA, qbB, kbA, kbB (dup'd bias columns)
            h = si % 2
            dst = q_f[h] if si < 2 else k_f[h]
            for tch in range(2):
                ps = ps_big.tile([P, 512], F32, tag="big")
                for e in range(4):
                    nc.tensor.matmul(
                        ps[:],
                        wsel(e)[:, h * P:(h + 1) * P],
                        xT[:, e, tch * 512:(tch + 1) * 512],
                        start=(e == 0),
                        stop=(e == 3),
                    )
                tmp = work.tile([P, 512], F32, tag="brelu")
                nc.scalar.activation(
                    tmp[:], ps[:], ACTF.Relu, bias=cf32[:, bcol:bcol + 1], scale=1.0
                )
                nc.vector.tensor_mul(
                    dst[:, tch * 512:(tch + 1) * 512],
                    tmp[:],
                    sc[:, tch * 512:(tch + 1) * 512],
                )

        # ---- stage C: sequence-layout v (ones col) and scaled k ------------
        # psum cols: 0:64 vA, 64:128 vB, 128:192 kA, 192:256 kB
        for ch in range(NCHUNK):
            ps = ps_big.tile([P, 256], F32, tag="big")
            nc.tensor.matmul(ps[:], ones_row[:], vkb[:], start=True, stop=False)
            for e in range(4):
                nc.tensor.matmul(ps[:], xT[:, e, ch * P:(ch + 1) * P], wvk(e),
                                 start=False, stop=(e == 3))
            # v: one strided copy for both heads + ones col
            nc.vector.tensor_copy(
                v_t[:, ch, :, 0:D],
                ps[:, 0:128].rearrange("p (h d) -> p h d", h=2),
            )
            nc.vector.memset(v_t[:, ch, :, D:D + 1], 1.0)
            # k_t: relu+scale on ACT (scale AP is per-partition; s,c > 0 so
            # relu(x)*s == relu(x*s))
            kc = ps[:, 128:256].rearrange("p (h d) -> p h d", h=2)
            nc.scalar.activation(k_t[:, ch, :, 0, :], kc, ACTF.Relu,
                                 scale=scol[:, ch:ch + 1])
            nc.scalar.activation(k_t[:, ch, :, 1, :], kc, ACTF.Relu,
                                 scale=ccol[:, ch:ch + 1])

        # ---- stage D1: per-chunk local states + prefix sum -----------------
        for ch in range(NCHUNK):
            psc = ps_po.tile([P, 2, D + 1], F32, tag="po130")
            for h in range(2):
                nc.tensor.matmul(psc[:, h, :], k_t[:, ch, h, :, :],
                                 v_t[:, ch, h, :], start=True, stop=True)
            nc.scalar.activation(Sc_sb[:, ch, :, :], psc[:], ACTF.Copy)
        nc.vector.tensor_copy(Spfx[:, 1], Sc_sb[:, 0])
        for ch in range(2, NCHUNK):
            nc.vector.tensor_add(Spfx[:, ch], Spfx[:, ch - 1], Sc_sb[:, ch - 1])

        # ---- stage D2: per-chunk attention ---------------------------------
        for ch in range(NCHUNK):
            cs = slice(ch * P, (ch + 1) * P)
            po = ps_po.tile([P, 2, D + 1], F32, tag="po130")
            for h in range(2):
                pss = ps_misc.tile([P, P], F32, tag="sq", bufs=2)
                nc.tensor.matmul(pss[:], k_f[h][:, cs], q_f[h][:, cs],
                                 start=True, stop=True)
                ms = work.tile([P, P], BF16, tag="ms")
                nc.vector.tensor_mul(ms[:], pss[:], mask[:])
                nc.tensor.matmul(po[:, h, :], ms[:], v_t[:, ch, h, :],
                                 start=True, stop=(ch == 0))
                if ch > 0:
                    nc.tensor.matmul(po[:, h, :], q_f[h][:, cs],
                                     Spfx[:, ch, h, :], start=False, stop=True)
            den = small.tile([P, 2], F32, tag="den")
            nc.vector.tensor_scalar(den[:], po[:, :, D], scalar1=EPS,
                                    scalar2=None, op0=ALU.max)
            rec = small.tile([P, 2], F32, tag="rec")
            nc.vector.reciprocal(rec[:], den[:])
            nc.vector.tensor_mul(
                attn[:, ch, :].rearrange("p (h d) -> p h d", h=2),
                po[:, :, 0:D],
                bcast(rec[:, :], [D]),
            )

        # ---- stage E: transpose attn (batched 4/bank) + output proj --------
        for g in range(2):
            tp = ps_misc.tile([P, 4, P], BF16, tag="tp", bufs=1)
            for i in range(4):
                nc.tensor.transpose(tp[:, i, :], attn[:, g * 4 + i, :], ident)
            nc.vector.tensor_copy(aT[:, g * 4:(g + 1) * 4, :], tp[:])
            for i in range(4):
                ch = g * 4 + i
                pso = ps_big.tile([P, E], F32, tag="big")
                nc.tensor.matmul(pso[:], aT[:, ch, :], outw, start=True, stop=True)
                osb = work.tile([P, E], F32, tag="osb")
                nc.scalar.activation(osb[:], pso[:], ACTF.Copy)
                nc.sync.dma_start(out=out_d[ch * P:(ch + 1) * P, :], in_=osb[:])

        for p in (ps_po, ps_misc, ps_big, small, work, persist):
            p.release()

    _split_multi_waits(nc)
    return nc


_PROG = {}


def _get_program():
    if "nc" not in _PROG:
        _PROG["nc"] = build_program()
    return _PROG["nc"]


def _prep_core_inputs(dev, query, q_w, q_b, k_w, k_b, v_w, v_b, out_w):
    n = dev // 4
    hA = 2 * (dev % 4)
    a, b = hA * D, (hA + 1) * D

    def dup(w, lo):
        wt = w[lo:lo + D, :].T  # (E, 64)
        return np.concatenate([wt, wt], axis=1)  # (E, 128)

    xT = np.ascontiguousarray(query[:, n, :].T.astype(np.float32))
    wq_f = np.concatenate([dup(q_w, a), dup(q_w, b)], axis=1)     # (E, 256)
    wk_f = np.concatenate([dup(k_w, a), dup(k_w, b)], axis=1)     # (E, 256)
    w_vk = np.concatenate(
        [v_w[a:a + D, :].T, v_w[b:b + D, :].T,
         k_w[a:a + D, :].T, k_w[b:b + D, :].T], axis=1)           # (E, 256)
    w_all = np.concatenate([wq_f, wk_f, w_vk], axis=1)            # (E, 768)
    outwT = np.concatenate([out_w[:, a:a + D].T, out_w[:, b:b + D].T], axis=0)
    wb16 = np.concatenate([outwT, np.eye(P, dtype=np.float32)], axis=1)

    idx = np.arange(1, L + 1, dtype=np.float64) * (np.pi / 2) / L
    s = np.sin(idx).astype(np.float32)
    c = np.cos(idx).astype(np.float32)
    sc_full = np.concatenate(
        [np.broadcast_to(s, (D, L)), np.broadcast_to(c, (D, L))], axis=0
    ).astype(np.float32)
    s_col = np.ascontiguousarray(s.reshape(NCHUNK, P).T)
    c_col = np.ascontiguousarray(c.reshape(NCHUNK, P).T)
    pi = np.arange(P)
    mask = (pi[:, None] <= pi[None, :]).astype(np.float32)
    qb_f = np.stack(
        [np.concatenate([q_b[a:a + D]] * 2), np.concatenate([q_b[b:b + D]] * 2)],
        axis=1).astype(np.float32)
    kb_f = np.stack(
        [np.concatenate([k_b[a:a + D]] * 2), np.concatenate([k_b[b:b + D]] * 2)],
        axis=1).astype(np.float32)
    cf32 = np.concatenate([sc_full, mask, s_col, c_col, qb_f, kb_f], axis=1)
    vkb = np.concatenate(
        [v_b[a:a + D], v_b[b:b + D], k_b[a:a + D], k_b[b:b + D]])
    row1 = np.concatenate(
        [vkb.astype(np.float32), np.ones(P, np.float32)]).reshape(1, 384)

    return {
        "xT": xT.astype(BF16NP),
        "w_all": np.ascontiguousarray(w_all).astype(BF16NP),
        "wb16": np.ascontiguousarray(wb16).astype(BF16NP),
        "cf32": np.ascontiguousarray(cf32.astype(np.float32)),
        "row1": row1.astype(BF16NP),
    }


def run(inputs, trace=False, trace_kwargs=None):
    nc = _get_program()
    in_maps = [
        _prep_core_inputs(
            d, inputs["query"], inputs["q_w"], inputs["q_b"], inputs["k_w"],
            inputs["k_b"], inputs["v_w"], inputs["v_b"], inputs["out_w"])
        for d in range(NCORES)
    ]
    res = bass_utils.run_bass_kernel_spmd(
        nc, in_maps, list(range(NCORES)), trace=trace,
        **(trace_kwargs or {}),
    )
    parts = [res.results[i]["out"] for i in range(NCORES)]
    out0 = parts[0] + parts[1] + parts[2] + parts[3]
    out1 = parts[4] + parts[5] + parts[6] + parts[7]
    out = np.stack([out0, out1], axis=1) + inputs["out_b"][None, None, :]
    return out.astype(np.float32), res


def kernel(**inputs) -> np.ndarray:
    out, _ = run(inputs, trace=False)
    return out



# revision 16
# speedup vs baseline: 1.1633x; 1.1633x over previous
"""CosformerAttention (causal linear attention) Trainium2 Bass kernel.

Full inputs in, full output out. Shards batch*heads over 8 NeuronCores:
device d handles sample n = d//4 and heads hA = 2*(d%4), hB = hA+1.

Lean dataflow (v2):
  - q/k projected UNDOUBLED feature-major (128 = 2h x 64 feats, L) - the
    cos/sin reweighting identity  q_i.k_j doubled == cos(th_i-th_j) q_i.k_j
    is folded into the causal mask, so intra-chunk scores contract over 64
    plain features per head.
  - doubled q (for the inter-chunk state matmul) built by a cheap
    PE dup-matmul (strided identity view) + one DVE scale by the sin/cos
    table.
  - v projected feature-major, then PE-transposed per chunk to seq-major;
    k seq-major likewise from a transpose of k_p (scaled s/c on ACT).
  - chunked causal linear attention with prefix-summed inter-chunk states;
    partial output projection over the 128 local features; host sums the
    4 per-sample bf16 partials in f32.
  - input DMA triggers are hoisted to the head of the program so the
    ~7us engine-init prologue overlaps the input load.

Self-contained: hardcodes L=1024, N=2, E=512, H=8 from the problem spec.
"""

import sys

if "/opt/trn_rl_repo" not in sys.path:
    sys.path.insert(0, "/opt/trn_rl_repo")

import numpy as np
import ml_dtypes

BF16NP = ml_dtypes.bfloat16

import concourse.bass as bass
import concourse.tile as tile
from concourse import mybir
import concourse.bass_utils as bass_utils
from concourse.vector_clock import ScopedClock

F32 = mybir.dt.float32
BF16 = mybir.dt.bfloat16
ALU = mybir.AluOpType
ACTF = mybir.ActivationFunctionType

L, N, E, H = 1024, 2, 512, 8
D = E // H          # 64 head dim
P = 128             # partitions / chunk size
NCHUNK = L // P     # 8
NCORES = 8
EPS = 1e-6

# wb (bf16 pack) column offsets
WQK = 0            # (4, 256) e-major [qA qB kA kB]
WV = 1024          # (4, 128) e-major [vA vB]
OUTW = 1536        # (512,)
IDENT = 2048       # (128,)
SC = 2176          # (1024,) rows 0:64 = sin, 64:128 = cos
DUP = 3200         # (2, 128) dup_h[p, f] = (p == h*64 + f%64)
WBCOLS = 3456
# cpack (f32) column offsets
MASK = 0           # (128,) cosmask
SCOL = 128         # (8,)
CCOL = 136         # (8,)
QB = 144
KB = 145
VB = 146
CPCOLS = 147


# ---------------------------------------------------------------------------
# This walrus build allows at most ONE semaphore wait per instruction.
# (a) Tile's tail drain carries the whole global clock: split it across
#     preceding SP nops.  (b) Skip the tail barriers + semaphore clearing --
#     the Bass preamble already dma_resets + sem_clears the entire kernel
#     semaphore range at program start, so end-of-kernel cleanup is
#     redundant and costs ~10us of EVSEM butterfly.
# ---------------------------------------------------------------------------
def _patched_drain_and_barrier(self, tick_clock, wait_clock):
    nc = self.nc
    nops = [nc.sync.nop() for _ in range(48)]
    drain_inst = nc.sync.drain()
    wait_clock.add_sem_waits(
        drain_inst.ins, ScopedClock({None: tick_clock.global_clock})
    )
    waits = list(drain_inst.ins.sync_info.on_wait or [])
    if len(waits) > 1:
        drain_inst.ins.sync_info.on_wait = [waits[-1]]
        SI = type(drain_inst.ins.sync_info)
        for nop, w in zip(nops, waits[:-1]):
            si = nop.ins.sync_info
            if si is None:
                nop.ins.sync_info = SI(on_wait=[w], on_update=[])
            else:
                si.on_wait = [w]
    nc.all_engine_barrier()
    popped = nc._tile_sem_poison_stack.pop()
    assert popped is self._sem_poison


tile.TileContext._drain_and_barrier = _patched_drain_and_barrier


def _split_multi_waits(nc):
    """Move excess sem waits onto preceding same-engine NoOps (engines
    execute strictly in order, so this is equivalent)."""
    k = 0
    for f in nc.m.functions:
        for bb in f.blocks:
            insts = list(bb.instructions)
            out, changed = [], False
            for inst in insts:
                si = inst.sync_info
                waits = list(si.on_wait) if (si is not None and si.on_wait) else []
                if len(waits) > 1 and "Unassigned" not in str(inst.engine):
                    for w in waits[:-1]:
                        nop = mybir.InstNoOp(name=f"wsplit-{k}", ins=[], outs=[])
                        k += 1
                        nop.engine = inst.engine
                        nop.sync_info = type(si)(on_wait=[w], on_update=[])
                        out.append(nop)
                    si.on_wait = [waits[-1]]
                    changed = True
                out.append(inst)
            if changed:
                bb.instructions = out


def _hoist_input_dmas(nc, n_inputs):
    """Move the first n_inputs InstDMACopy (the input loads, which have no
    waits) from the tile block to the head of the main block, so the input
    DMA overlaps the engine-init prologue."""
    blocks = [bb for f in nc.m.functions for bb in f.blocks]
    main = next(bb for bb in blocks if bb.name == "main")
    tb = next(bb for bb in blocks if bb.name.startswith("tile_context"))
    moved, rest = [], []
    for inst in tb.instructions:
        if (len(moved) < n_inputs and type(inst).__name__ == "InstDMACopy"
                and not (inst.sync_info and inst.sync_info.on_wait)):
            moved.append(inst)
        else:
            rest.append(inst)
    assert len(moved) == n_inputs, f"found {len(moved)} input DMAs"
    tb.instructions = rest
    main.instructions = moved + list(main.instructions)


def bcast(ap, dims):
    """Append broadcast (step 0) free dims to an AP."""
    return bass.AP(tensor=ap.tensor, offset=ap.offset,
                   ap=list(ap.ap) + [[0, d] for d in dims])


def build_program():
    nc = bass.Bass("TRN2", target_bir_lowering=False)

    # ---- DRAM I/O (packed; layouts match SBUF tiles exactly) ---------------
    xT_d = nc.dram_tensor("xT", [P, 4, L], BF16, kind="ExternalInput").ap()
    wb_d = nc.dram_tensor("wb", [P, WBCOLS], BF16, kind="ExternalInput").ap()
    cp_d = nc.dram_tensor("cp", [P, CPCOLS], F32, kind="ExternalInput").ap()
    out_d = nc.dram_tensor("out", [L, E], BF16, kind="ExternalOutput").ap()

    with tile.TileContext(nc) as tc:
        persist = tc.alloc_tile_pool(name="persist", bufs=1)
        work = tc.alloc_tile_pool(name="work", bufs=3)
        small = tc.alloc_tile_pool(name="small", bufs=4)
        ps_big = tc.alloc_tile_pool(name="ps_big", bufs=2, space="PSUM")
        ps_sq = tc.alloc_tile_pool(name="ps_sq", bufs=2, space="PSUM")
        ps_tp = tc.alloc_tile_pool(name="ps_tp", bufs=2, space="PSUM")
        ps_po = tc.alloc_tile_pool(name="ps_po", bufs=2, space="PSUM")

        # ---- batched input loads (hoisted to program head post-build) ------
        xT = persist.tile([P, 4, L], BF16, tag="xT", name="xT")
        nc.sync.dma_start(out=xT[:], in_=xT_d)
        wb = persist.tile([P, WBCOLS], BF16, tag="wb", name="wb")
        nc.sync.dma_start(out=wb[:], in_=wb_d)
        cp = persist.tile([P, CPCOLS], F32, tag="cp", name="cp")
        nc.sync.dma_start(out=cp[:], in_=cp_d)

        identv = wb[:, IDENT:IDENT + P]
        outw = wb[:, OUTW:OUTW + E]
        sc = wb[:, SC:SC + L]
        cosmask = cp[:, MASK:MASK + P]

        # persistent activations
        q_p = persist.tile([P, L], BF16, tag="qp", name="qp")
        k_p = persist.tile([P, L], BF16, tag="kp", name="kp")
        v_fm = persist.tile([P, L], BF16, tag="vfm", name="vfm")
        q_f = [persist.tile([P, L], BF16, tag=f"qf{h}", name=f"qf{h}")
               for h in range(2)]
        # k_t: [ch, head, sc, d] sequence-layout scaled k
        k_t = persist.tile([P, NCHUNK, 2, 2, D], BF16, tag="kt", name="kt")
        # v_t: [ch, head, d+1] with ones column
        v_t = persist.tile([P, NCHUNK, 2, D + 1], BF16, tag="vt", name="vt")
        attn = persist.tile([P, NCHUNK, P], BF16, tag="attn", name="attn")
        Sc_sb = persist.tile([P, NCHUNK, 2, D + 1], BF16, tag="scsb", name="scsb")
        Spfx = persist.tile([P, NCHUNK, 2, D + 1], BF16, tag="spfx", name="spfx")

        nc.vector.memset(v_t[:, :, :, D:D + 1], 1.0)

        # ---- stage B: feature-major q/k/v projections ----------------------
        def proj(wlo, wstep, wwid, bias_col, actf, dst):
            for tch in range(2):
                ts = slice(tch * 512, (tch + 1) * 512)
                ps = ps_big.tile([P, 512], F32, tag="big")
                for e in range(4):
                    nc.tensor.matmul(
                        ps[:], wb[:, wlo + e * wstep: wlo + e * wstep + wwid],
                        xT[:, e, ts], start=(e == 0), stop=(e == 3))
                nc.scalar.activation(dst[:, ts], ps[:], actf,
                                     bias=cp[:, bias_col:bias_col + 1],
                                     scale=1.0)

        proj(WQK, 256, 128, QB, ACTF.Relu, q_p)
        proj(WQK + 128, 256, 128, KB, ACTF.Relu, k_p)

        # dup q -> doubled layout, scale by sin/cos table
        for h in range(2):
            for tch in range(2):
                ts = slice(tch * 512, (tch + 1) * 512)
                psd = ps_big.tile([P, 512], F32, tag="big")
                nc.tensor.matmul(psd[:], wb[:, DUP + h * P:DUP + (h + 1) * P],
                                 q_p[:, ts], start=True, stop=True)
                nc.vector.tensor_mul(q_f[h][:, ts], psd[:], sc[:, ts])

        proj(WV, 128, 128, VB, ACTF.Identity, v_fm)

        # ---- stage C: per-chunk transposes to sequence layout --------------
        for ch in range(NCHUNK):
            cs = slice(ch * P, (ch + 1) * P)
            pst = ps_tp.tile([P, P], BF16, tag="tp")
            nc.tensor.transpose(pst[:], k_p[:, cs], identv)
            kc = pst[:].rearrange("p (h d) -> p h d", h=2)
            nc.scalar.activation(k_t[:, ch, :, 0, :], kc, ACTF.Relu,
                                 scale=cp[:, SCOL + ch:SCOL + ch + 1])
            nc.scalar.activation(k_t[:, ch, :, 1, :], kc, ACTF.Relu,
                                 scale=cp[:, CCOL + ch:CCOL + ch + 1])
        for ch in range(NCHUNK):
            cs = slice(ch * P, (ch + 1) * P)
            pst = ps_tp.tile([P, P], BF16, tag="tp")
            nc.tensor.transpose(pst[:], v_fm[:, cs], identv)
            nc.vector.tensor_copy(
                v_t[:, ch, :, 0:D],
                pst[:].rearrange("p (h d) -> p h d", h=2))

        # ---- stage D1: per-chunk local states + prefix sum -----------------
        for ch in range(NCHUNK):
            psc = ps_po.tile([P, 2, D + 1], F32, tag="po130")
            for h in range(2):
                nc.tensor.matmul(psc[:, h, :], k_t[:, ch, h, :, :],
                                 v_t[:, ch, h, :], start=True, stop=True)
            nc.scalar.activation(Sc_sb[:, ch, :, :], psc[:], ACTF.Copy)
        nc.vector.tensor_copy(Spfx[:, 1], Sc_sb[:, 0])
        for ch in range(2, NCHUNK):
            nc.vector.tensor_add(Spfx[:, ch], Spfx[:, ch - 1], Sc_sb[:, ch - 1])

        # ---- stage D2 + E: per-chunk attention, lag-one output proj --------
        osb_ref = [None]

        def stage_e(ch):
            g, j = ch // 2, ch % 2
            pst = ps_tp.tile([P, P], BF16, tag="tp")
            nc.tensor.transpose(pst[:], attn[:, ch, :], identv)
            aTw = work.tile([P, P], BF16, tag="aT")
            nc.vector.tensor_copy(aTw[:], pst[:])
            pso = ps_big.tile([P, E], F32, tag="big")
            nc.tensor.matmul(pso[:], aTw[:], outw, start=True, stop=True)
            if j == 0:
                osb_ref[0] = work.tile([P, 2, E], BF16, tag="osb", name="osb")
            if ch % 2 == 0:
                nc.scalar.activation(osb_ref[0][:, j, :], pso[:], ACTF.Copy)
            else:
                nc.vector.tensor_copy(osb_ref[0][:, j, :], pso[:])
            if j == 1:
                nc.sync.dma_start(
                    out=out_d[g * 2 * P:(g + 1) * 2 * P, :].rearrange(
                        "(j p) e -> p j e", p=P),
                    in_=osb_ref[0][:])

        for ch in range(NCHUNK):
            cs = slice(ch * P, (ch + 1) * P)
            po = ps_po.tile([P, 2, D + 1], F32, tag="po130")
            ms = []
            for h in range(2):
                pss = ps_sq.tile([P, P], F32, tag="sq")
                nc.tensor.matmul(pss[:], k_p[h * D:(h + 1) * D, cs],
                                 q_p[h * D:(h + 1) * D, cs],
                                 start=True, stop=True)
                m = work.tile([P, P], BF16, tag="ms")
                nc.vector.tensor_mul(m[:], pss[:], cosmask)
                ms.append(m)
            for h in range(2):
                nc.tensor.matmul(po[:, h, :], ms[h][:], v_t[:, ch, h, :],
                                 start=True, stop=(ch == 0))
                if ch > 0:
                    nc.tensor.matmul(po[:, h, :], q_f[h][:, cs],
                                     Spfx[:, ch, h, :], start=False, stop=True)
            den = small.tile([P, 2], F32, tag="den")
            nc.vector.tensor_scalar(den[:], po[:, :, D], scalar1=EPS,
                                    scalar2=None, op0=ALU.max)
            rec = small.tile([P, 2], F32, tag="rec")
            nc.vector.reciprocal(rec[:], den[:])
            nc.vector.tensor_mul(
                attn[:, ch, :].rearrange("p (h d) -> p h d", h=2),
                po[:, :, 0:D],
                bcast(rec[:, :], [D]),
            )
            if ch >= 1:
                stage_e(ch - 1)
        stage_e(NCHUNK - 1)

        for p in (ps_po, ps_tp, ps_sq, ps_big, small, work, persist):
            p.release()

    _split_multi_waits(nc)
    _hoist_input_dmas(nc, 3)
    return nc


_PROG = {}


def _get_program():
    if "nc" not in _PROG:
        _PROG["nc"] = build_program()
    return _PROG["nc"]


_CONST = {}


def _const_tables():
    if not _CONST:
        idx = np.arange(1, L + 1, dtype=np.float64) * (np.pi / 2) / L
        s, c = np.sin(idx), np.cos(idx)
        _CONST["sc"] = np.concatenate(
            [np.broadcast_to(s, (D, L)), np.broadcast_to(c, (D, L))],
            axis=0).astype(BF16NP)
        jj, ii = np.meshgrid(np.arange(P), np.arange(P), indexing="ij")
        _CONST["cosmask"] = (
            np.cos((np.pi / 2) * (ii - jj) / L) * (jj <= ii)).astype(np.float32)
        _CONST["s_col"] = np.ascontiguousarray(
            s.reshape(NCHUNK, P).T).astype(np.float32)
        _CONST["c_col"] = np.ascontiguousarray(
            c.reshape(NCHUNK, P).T).astype(np.float32)
        _CONST["ident"] = np.eye(P, dtype=np.float32)
        pp, ff = np.meshgrid(np.arange(P), np.arange(P), indexing="ij")
        dups = [(pp == h * D + ff % D).astype(np.float32) for h in range(2)]
        _CONST["dup"] = np.concatenate(dups, axis=1)  # (128, 256)
    return _CONST


def _prep_core_inputs(dev, query, q_w, q_b, k_w, k_b, v_w, v_b, out_w):
    n = dev // 4
    hA = 2 * (dev % 4)
    a, b = hA * D, (hA + 1) * D
    cst = _const_tables()

    def pack_pe(w):
        # (128 feats, E) weight rows -> (p, e, f) stationary layout
        sel = np.concatenate([w[a:a + D, :], w[b:b + D, :]], axis=0)  # (128, E)
        return np.ascontiguousarray(
            sel.T.reshape(4, P, P).transpose(1, 0, 2))  # (p, e, f)

    x = query[:, n, :].astype(np.float32)  # (L, E)
    xT = np.ascontiguousarray(x.T.reshape(4, P, L).transpose(1, 0, 2))

    wqk = np.concatenate([pack_pe(q_w), pack_pe(k_w)], axis=2)  # (p, 4, 256)
    wv = pack_pe(v_w)                                           # (p, 4, 128)
    outwT = np.concatenate(
        [out_w[:, a:a + D].T, out_w[:, b:b + D].T], axis=0)     # (128, 512)
    wbp = np.concatenate(
        [wqk.reshape(P, 1024), wv.reshape(P, 512), outwT,
         cst["ident"], cst["sc"].astype(np.float32), cst["dup"]],
        axis=1)                                                 # (128, 3456)

    def bias_col(v):
        return np.concatenate([v[a:a + D], v[b:b + D]]).reshape(P, 1)

    cpk = np.concatenate(
        [cst["cosmask"], cst["s_col"], cst["c_col"],
         bias_col(q_b), bias_col(k_b), bias_col(v_b)],
        axis=1).astype(np.float32)                              # (128, 147)

    return {
        "xT": xT.astype(BF16NP),
        "wb": np.ascontiguousarray(wbp).astype(BF16NP),
        "cp": np.ascontiguousarray(cpk),
    }


def run(inputs, trace=False, trace_kwargs=None):
    nc = _get_program()
    in_maps = [
        _prep_core_inputs(
            d, inputs["query"], inputs["q_w"], inputs["q_b"], inputs["k_w"],
            inputs["k_b"], inputs["v_w"], inputs["v_b"], inputs["out_w"])
        for d in range(NCORES)
    ]
    res = bass_utils.run_bass_kernel_spmd(
        nc, in_maps, list(range(NCORES)), trace=trace,
        **(trace_kwargs or {}),
    )
    parts = [res.results[i]["out"].astype(np.float32) for i in range(NCORES)]
    out0 = parts[0] + parts[1] + parts[2] + parts[3]
    out1 = parts[4] + parts[5] + parts[6] + parts[7]
    out = np.stack([out0, out1], axis=1) + inputs["out_b"][None, None, :]
    return out.astype(np.float32), res


def kernel(**inputs) -> np.ndarray:
    out, _ = run(inputs, trace=False)
    return out


# revision 21
# speedup vs baseline: 1.2943x; 1.1126x over previous
"""CosformerAttention (causal linear attention) Trainium2 Bass kernel.

Full inputs in, full output out. Shards batch*heads over 8 NeuronCores:
device d handles sample n = d//4 and heads hA = 2*(d%4), hB = hA+1.

Lean dataflow (v3):
  - q/k projected UNDOUBLED feature-major (128 = 2h x 64 feats, L) - the
    cos/sin reweighting identity (doubled q_i . doubled k_j ==
    cos(th_i-th_j) * q_i.k_j) is folded into the causal mask, so
    intra-chunk scores contract over 64 plain features per head.
  - doubled q (inter-chunk state matmul stationary) built by a PE
    dup-matmul + one DVE scale by the sin/cos table.
  - v projected feature-major, then PE-transposed per chunk to seq-major;
    k seq-major likewise (s/c scaling split across ACT and DVE).
  - prefix sum of chunk states on the otherwise-idle GpSimd.
  - inputs split/ordered so the first projection starts ~2.5us after the
    fixed ~6.8us engine-init prologue (DMA triggers hoisted to program
    head); bf16 output partials, host sums 4 per-sample partials in f32.

Self-contained: hardcodes L=1024, N=2, E=512, H=8 from the problem spec.
"""

import sys

if "/opt/trn_rl_repo" not in sys.path:
    sys.path.insert(0, "/opt/trn_rl_repo")

import numpy as np
import ml_dtypes

BF16NP = ml_dtypes.bfloat16

import concourse.bass as bass
import concourse.tile as tile
from concourse import mybir
import concourse.bass_utils as bass_utils
from concourse.vector_clock import ScopedClock

F32 = mybir.dt.float32
BF16 = mybir.dt.bfloat16
ALU = mybir.AluOpType
ACTF = mybir.ActivationFunctionType

L, N, E, H = 1024, 2, 512, 8
D = E // H          # 64 head dim
P = 128             # partitions / chunk size
NCHUNK = L // P     # 8
NCORES = 8
EPS = 1e-6
TCH = 512           # seq half handled per xT tile

# wbA (bf16) column offsets: stationaries needed by the first matmuls
WQK = 0            # (4, 256) e-major [qA qB kA kB]
DUP = 1024         # (2, 128) dup_h[p, f] = (p == h*64 + f%64)
WACOLS = 1280
# wbB (bf16) column offsets
WV = 0             # (4, 128) e-major [vA vB]
OUTW = 512         # (512,)
IDENT = 1024       # (128,)
SC = 1152          # (1024,) rows 0:64 = sin, 64:128 = cos
WBCOLS = 2176
# cpack (f32) column offsets
MASK = 0           # (128,) cosmask
SCOL = 128         # (8,)
CCOL = 136         # (8,)
QB = 144
KB = 145
VB = 146
CPCOLS = 147


# ---------------------------------------------------------------------------
# This walrus build allows at most ONE semaphore wait per instruction.
# (a) Tile's tail drain carries the whole global clock: split it across
#     preceding SP nops.  (b) Skip the tail barriers + semaphore clearing --
#     the Bass preamble already dma_resets + sem_clears the entire kernel
#     semaphore range at program start, so end-of-kernel cleanup is
#     redundant and costs ~10us of EVSEM butterfly.
# ---------------------------------------------------------------------------
def _patched_drain_and_barrier(self, tick_clock, wait_clock):
    nc = self.nc
    nops = [nc.sync.nop() for _ in range(48)]
    drain_inst = nc.sync.drain()
    wait_clock.add_sem_waits(
        drain_inst.ins, ScopedClock({None: tick_clock.global_clock})
    )
    waits = list(drain_inst.ins.sync_info.on_wait or [])
    if len(waits) > 1:
        drain_inst.ins.sync_info.on_wait = [waits[-1]]
        SI = type(drain_inst.ins.sync_info)
        for nop, w in zip(nops, waits[:-1]):
            si = nop.ins.sync_info
            if si is None:
                nop.ins.sync_info = SI(on_wait=[w], on_update=[])
            else:
                si.on_wait = [w]
    nc.all_engine_barrier()
    popped = nc._tile_sem_poison_stack.pop()
    assert popped is self._sem_poison


tile.TileContext._drain_and_barrier = _patched_drain_and_barrier


def _split_multi_waits(nc):
    """Move excess sem waits onto preceding same-engine NoOps (engines
    execute strictly in order, so this is equivalent)."""
    k = 0
    for f in nc.m.functions:
        for bb in f.blocks:
            insts = list(bb.instructions)
            out, changed = [], False
            for inst in insts:
                si = inst.sync_info
                waits = list(si.on_wait) if (si is not None and si.on_wait) else []
                if len(waits) > 1 and "Unassigned" not in str(inst.engine):
                    for w in waits[:-1]:
                        nop = mybir.InstNoOp(name=f"wsplit-{k}", ins=[], outs=[])
                        k += 1
                        nop.engine = inst.engine
                        nop.sync_info = type(si)(on_wait=[w], on_update=[])
                        out.append(nop)
                    si.on_wait = [waits[-1]]
                    changed = True
                out.append(inst)
            if changed:
                bb.instructions = out


def _hoist_input_dmas(nc, n_inputs):
    """Move the first n_inputs InstDMACopy (the input loads, which have no
    waits) from the tile block to the head of the main block, so the input
    DMA overlaps the engine-init prologue."""
    blocks = [bb for f in nc.m.functions for bb in f.blocks]
    main = next(bb for bb in blocks if bb.name == "main")
    tb = next(bb for bb in blocks if bb.name.startswith("tile_context"))
    moved, rest = [], []
    for inst in tb.instructions:
        if (len(moved) < n_inputs and type(inst).__name__ == "InstDMACopy"
                and not (inst.sync_info and inst.sync_info.on_wait)):
            moved.append(inst)
        else:
            rest.append(inst)
    assert len(moved) == n_inputs, f"found {len(moved)} input DMAs"
    tb.instructions = rest
    main.instructions = moved + list(main.instructions)


def bcast(ap, dims):
    """Append broadcast (step 0) free dims to an AP."""
    return bass.AP(tensor=ap.tensor, offset=ap.offset,
                   ap=list(ap.ap) + [[0, d] for d in dims])


def mid_bcast(ap, n):
    """Insert a step-0 dim of size n between partition dim and free dims."""
    return bass.AP(tensor=ap.tensor, offset=ap.offset,
                   ap=[ap.ap[0], [0, n]] + list(ap.ap[1:]))


def build_program(hoist=True):
    nc = bass.Bass("TRN2", target_bir_lowering=False)

    # ---- DRAM I/O (layouts match SBUF tiles exactly) -----------------------
    wa_d = nc.dram_tensor("wa", [P, WACOLS], BF16, kind="ExternalInput").ap()
    x0_d = nc.dram_tensor("x0", [P, 4, TCH], BF16, kind="ExternalInput").ap()
    wb_d = nc.dram_tensor("wb", [P, WBCOLS], BF16, kind="ExternalInput").ap()
    x1_d = nc.dram_tensor("x1", [P, 4, TCH], BF16, kind="ExternalInput").ap()
    cp_d = nc.dram_tensor("cp", [P, CPCOLS], F32, kind="ExternalInput").ap()
    out_d = nc.dram_tensor("out", [L, E], BF16, kind="ExternalOutput").ap()

    with tile.TileContext(nc) as tc:
        persist = tc.alloc_tile_pool(name="persist", bufs=1)
        work = tc.alloc_tile_pool(name="work", bufs=3)
        small = tc.alloc_tile_pool(name="small", bufs=4)
        ps_big = tc.alloc_tile_pool(name="ps_big", bufs=2, space="PSUM")
        ps_sq = tc.alloc_tile_pool(name="ps_sq", bufs=2, space="PSUM")
        ps_tp = tc.alloc_tile_pool(name="ps_tp", bufs=2, space="PSUM")
        ps_po = tc.alloc_tile_pool(name="ps_po", bufs=2, space="PSUM")

        # ---- input loads (hoisted to program head post-build), in the
        # order the compute consumes them ------------------------------------
        wa = persist.tile([P, WACOLS], BF16, tag="wa", name="wa")
        nc.sync.dma_start(out=wa[:], in_=wa_d)
        xT = [persist.tile([P, 4, TCH], BF16, tag=f"x{t}", name=f"x{t}")
              for t in range(2)]
        nc.sync.dma_start(out=xT[0][:], in_=x0_d)
        wb = persist.tile([P, WBCOLS], BF16, tag="wb", name="wb")
        nc.sync.dma_start(out=wb[:], in_=wb_d)
        nc.sync.dma_start(out=xT[1][:], in_=x1_d)
        cp = persist.tile([P, CPCOLS], F32, tag="cp", name="cp")
        nc.sync.dma_start(out=cp[:], in_=cp_d)

        identv = wb[:, IDENT:IDENT + P]
        outw = wb[:, OUTW:OUTW + E]
        cosmask = cp[:, MASK:MASK + P]

        # persistent activations (split per seq-half for finer DMA overlap)
        q_p = [persist.tile([P, TCH], BF16, tag=f"qp{t}", name=f"qp{t}")
               for t in range(2)]
        k_p = [persist.tile([P, TCH], BF16, tag=f"kp{t}", name=f"kp{t}")
               for t in range(2)]
        v_fm = [persist.tile([P, TCH], BF16, tag=f"vfm{t}", name=f"vfm{t}")
                for t in range(2)]
        q_f = [persist.tile([P, L], BF16, tag=f"qf{h}", name=f"qf{h}")
               for h in range(2)]
        # k_t: [ch, head, sc, d] sequence-layout scaled k
        k_t = persist.tile([P, NCHUNK, 2, 2, D], BF16, tag="kt", name="kt")
        # v_t: [ch, head, d+1] with ones column
        v_t = persist.tile([P, NCHUNK, 2, D + 1], BF16, tag="vt", name="vt")
        attn = persist.tile([P, NCHUNK, P], BF16, tag="attn", name="attn")
        Sc_sb = persist.tile([P, NCHUNK, 2, D + 1], BF16, tag="scsb", name="scsb")
        Spfx = persist.tile([P, NCHUNK, 2, D + 1], BF16, tag="spfx", name="spfx")

        nc.vector.memset(v_t[:, :, :, D:D + 1], 1.0)

        # ---- stage B: feature-major q/k/v projections ----------------------
        def proj(t, wlo, bias_col, actf, dst):
            ps = ps_big.tile([P, TCH], F32, tag="big", name="ps")
            for e in range(4):
                nc.tensor.matmul(
                    ps[:], wa[:, WQK + e * 256 + wlo: WQK + e * 256 + wlo + P]
                    if wlo < 256 else
                    wb[:, WV + e * P: WV + (e + 1) * P],
                    xT[t][:, e, :], start=(e == 0), stop=(e == 3))
            nc.scalar.activation(dst[t][:], ps[:], actf,
                                 bias=cp[:, bias_col:bias_col + 1], scale=1.0)

        def dup_q(t):
            for h in range(2):
                psd = ps_big.tile([P, TCH], F32, tag="big", name="psd")
                nc.tensor.matmul(psd[:], wa[:, DUP + h * P:DUP + (h + 1) * P],
                                 q_p[t][:], start=True, stop=True)
                nc.vector.tensor_mul(
                    q_f[h][:, t * TCH:(t + 1) * TCH], psd[:],
                    wb[:, SC + t * TCH:SC + (t + 1) * TCH])

        for t in range(2):
            proj(t, 0, QB, ACTF.Relu, q_p)       # q
            proj(t, 128, KB, ACTF.Relu, k_p)     # k
            dup_q(t)
            proj(t, 999, VB, ACTF.Identity, v_fm)  # v (wlo>=256 -> wv)

        # ---- stage C + D1 fused per chunk ----------------------------------
        for ch in range(NCHUNK):
            t, cs = ch // 4, slice((ch % 4) * P, (ch % 4 + 1) * P)
            ptk = ps_tp.tile([P, P], BF16, tag="tp", name="ptk")
            nc.tensor.transpose(ptk[:], k_p[t][:, cs], identv)
            kc = ptk[:].rearrange("p (h d) -> p h d", h=2)
            nc.scalar.activation(k_t[:, ch, :, 0, :], kc, ACTF.Relu,
                                 scale=cp[:, SCOL + ch:SCOL + ch + 1])
            nc.scalar.activation(k_t[:, ch, :, 1, :], kc, ACTF.Relu,
                                 scale=cp[:, CCOL + ch:CCOL + ch + 1])
            ptv = ps_tp.tile([P, P], BF16, tag="tp", name="ptv")
            nc.tensor.transpose(ptv[:], v_fm[t][:, cs], identv)
            nc.vector.tensor_copy(
                v_t[:, ch, :, 0:D],
                ptv[:].rearrange("p (h d) -> p h d", h=2))
            psc = ps_po.tile([P, 2, D + 1], F32, tag="po130", name="psc")
            for h in range(2):
                nc.tensor.matmul(psc[:, h, :], k_t[:, ch, h, :, :],
                                 v_t[:, ch, h, :], start=True, stop=True)
            nc.scalar.activation(Sc_sb[:, ch, :, :], psc[:], ACTF.Copy)
            if ch == 1:
                nc.vector.tensor_copy(Spfx[:, 1], Sc_sb[:, 0])
            elif ch > 1:
                nc.vector.tensor_add(Spfx[:, ch], Spfx[:, ch - 1],
                                     Sc_sb[:, ch - 1])

        # ---- stage D2 + E: per-chunk attention, lag-one output proj --------
        osb_ref = [None]

        def stage_e_head(ch):
            pst = ps_tp.tile([P, P], BF16, tag="tp", name="pst")
            nc.tensor.transpose(pst[:], attn[:, ch, :], identv)
            aTw = work.tile([P, P], BF16, tag="aT", name="aTw")
            nc.vector.tensor_copy(aTw[:], pst[:])
            return aTw

        def stage_e_tail(ch, aTw):
            g, j = ch // 2, ch % 2
            pso = ps_big.tile([P, E], F32, tag="big", name="pso")
            nc.tensor.matmul(pso[:], aTw[:], outw, start=True, stop=True)
            if j == 0:
                osb_ref[0] = work.tile([P, 2, E], BF16, tag="osb", name="osb")
            nc.scalar.activation(osb_ref[0][:, j, :], pso[:], ACTF.Copy)
            if j == 1:
                nc.sync.dma_start(
                    out=out_d[g * 2 * P:(g + 1) * 2 * P, :].rearrange(
                        "(j p) e -> p j e", p=P),
                    in_=osb_ref[0][:])

        aT_prev = None
        for ch in range(NCHUNK):
            t, cs = ch // 4, slice((ch % 4) * P, (ch % 4 + 1) * P)
            ms = work.tile([P, 2, P], BF16, tag="ms", name="ms")
            for h in range(2):
                pss = ps_sq.tile([P, P], F32, tag="sq", name="pss")
                nc.tensor.matmul(pss[:], k_p[t][h * D:(h + 1) * D, cs],
                                 q_p[t][h * D:(h + 1) * D, cs],
                                 start=True, stop=True)
                nc.vector.tensor_mul(ms[:, h, :], pss[:], cosmask)
            if ch >= 1:
                aT_prev = stage_e_head(ch - 1)
            po = ps_po.tile([P, 2, D + 1], F32, tag="po130", name="po")
            for h in range(2):
                nc.tensor.matmul(po[:, h, :], ms[:, h, :], v_t[:, ch, h, :],
                                 start=True, stop=(ch == 0))
                if ch > 0:
                    nc.tensor.matmul(po[:, h, :], q_f[h][:, ch * P:(ch + 1) * P],
                                     Spfx[:, ch, h, :], start=False, stop=True)
            if ch >= 1:
                stage_e_tail(ch - 1, aT_prev)
            den = small.tile([P, 2], F32, tag="den", name="den")
            nc.vector.tensor_scalar(den[:], po[:, :, D], scalar1=EPS,
                                    scalar2=None, op0=ALU.max)
            rec = small.tile([P, 2], F32, tag="rec", name="rec")
            nc.vector.reciprocal(rec[:], den[:])
            nc.vector.tensor_mul(
                attn[:, ch, :].rearrange("p (h d) -> p h d", h=2),
                po[:, :, 0:D],
                bcast(rec[:, :], [D]),
            )
        stage_e_tail(NCHUNK - 1, stage_e_head(NCHUNK - 1))

        for p in (ps_po, ps_tp, ps_sq, ps_big, small, work, persist):
            p.release()

    _split_multi_waits(nc)
    if hoist:
        _hoist_input_dmas(nc, 5)
    return nc


_PROG = {}


def _get_program():
    if "nc" not in _PROG:
        _PROG["nc"] = build_program()
    return _PROG["nc"]


_CONST = {}


def _const_tables():
    if not _CONST:
        idx = np.arange(1, L + 1, dtype=np.float64) * (np.pi / 2) / L
        s, c = np.sin(idx), np.cos(idx)
        _CONST["sc"] = np.concatenate(
            [np.broadcast_to(s, (D, L)), np.broadcast_to(c, (D, L))],
            axis=0).astype(BF16NP).astype(np.float32)
        jj, ii = np.meshgrid(np.arange(P), np.arange(P), indexing="ij")
        _CONST["cosmask"] = (
            np.cos((np.pi / 2) * (ii - jj) / L) * (jj <= ii)).astype(np.float32)
        _CONST["s_col"] = np.ascontiguousarray(
            s.reshape(NCHUNK, P).T).astype(np.float32)
        _CONST["c_col"] = np.ascontiguousarray(
            c.reshape(NCHUNK, P).T).astype(np.float32)
        _CONST["ident"] = np.eye(P, dtype=np.float32)
        pp, ff = np.meshgrid(np.arange(P), np.arange(P), indexing="ij")
        dups = [(pp == h * D + ff % D).astype(np.float32) for h in range(2)]
        _CONST["dup"] = np.concatenate(dups, axis=1)  # (128, 256)
    return _CONST


def _prep_core_inputs(dev, query, q_w, q_b, k_w, k_b, v_w, v_b, out_w):
    n = dev // 4
    hA = 2 * (dev % 4)
    a, b = hA * D, (hA + 1) * D
    cst = _const_tables()

    def pack_pe(w):
        # (128 feats, E) weight rows -> (p, e, f) stationary layout
        sel = np.concatenate([w[a:a + D, :], w[b:b + D, :]], axis=0)  # (128, E)
        return np.ascontiguousarray(
            sel.T.reshape(4, P, P).transpose(1, 0, 2))  # (p, e, f)

    x = query[:, n, :].astype(np.float32)  # (L, E)
    xT = np.ascontiguousarray(x.T.reshape(4, P, L).transpose(1, 0, 2))

    wqk = np.concatenate([pack_pe(q_w), pack_pe(k_w)], axis=2)  # (p, 4, 256)
    wa = np.concatenate([wqk.reshape(P, 1024), cst["dup"]], axis=1)
    wv = pack_pe(v_w)                                           # (p, 4, 128)
    outwT = np.concatenate(
        [out_w[:, a:a + D].T, out_w[:, b:b + D].T], axis=0)     # (128, 512)
    wbp = np.concatenate(
        [wv.reshape(P, 512), outwT, cst["ident"], cst["sc"]], axis=1)

    def bias_col(v):
        return np.concatenate([v[a:a + D], v[b:b + D]]).reshape(P, 1)

    cpk = np.concatenate(
        [cst["cosmask"], cst["s_col"], cst["c_col"],
         bias_col(q_b), bias_col(k_b), bias_col(v_b)],
        axis=1).astype(np.float32)                              # (128, 147)

    return {
        "wa": np.ascontiguousarray(wa).astype(BF16NP),
        "x0": np.ascontiguousarray(xT[:, :, :TCH]).astype(BF16NP),
        "wb": np.ascontiguousarray(wbp).astype(BF16NP),
        "x1": np.ascontiguousarray(xT[:, :, TCH:]).astype(BF16NP),
        "cp": np.ascontiguousarray(cpk),
    }


def run(inputs, trace=False, trace_kwargs=None):
    nc = _get_program()
    in_maps = [
        _prep_core_inputs(
            d, inputs["query"], inputs["q_w"], inputs["q_b"], inputs["k_w"],
            inputs["k_b"], inputs["v_w"], inputs["v_b"], inputs["out_w"])
        for d in range(NCORES)
    ]
    res = bass_utils.run_bass_kernel_spmd(
        nc, in_maps, list(range(NCORES)), trace=trace,
        **(trace_kwargs or {}),
    )
    parts = [res.results[i]["out"].astype(np.float32) for i in range(NCORES)]
    out0 = parts[0] + parts[1] + parts[2] + parts[3]
    out1 = parts[4] + parts[5] + parts[6] + parts[7]
    out = np.stack([out0, out1], axis=1) + inputs["out_b"][None, None, :]
    return out.astype(np.float32), res


def kernel(**inputs) -> np.ndarray:
    out, _ = run(inputs, trace=False)
    return out
